# revision 1
# baseline (speedup 1.0000x reference)
"""Trainium2 Bass kernel for nn_Diffusion_3418793968193 (gnn_message_passing).

Sharding: channel-sliced model parallelism over 8 NeuronCores.
 - Activations (y) are replicated; the big channel-mixing weights
   (conv_w / proj_w / out_w / htp_w) are sliced 256 rows per core on the host.
 - Per temporal layer: conv is column-parallel (each core computes its 256
   output channels against the full 2048-channel input), the h slices are
   AllGathered, proj is row-sliced, and the blk residual slices are
   AllGathered then added into the replicated y on every core.
 - GAT: each core computes attention rows for its 256 destination nodes
   (for all 8 batch samples), the Y slices are AllGathered, and the output
   1x1 conv is row-sliced again.
 - The GRU context encoder is replicated (tiny FLOPs); its 96-step
   recurrence overlaps the DMA/PE-bound bulk work.
 - Heavy matmuls run in float32r (full PE rate at moving-dim >= 256).
The program is identical on all cores (SPMD): every rank-specific value
arrives via host-sliced inputs or collective routing, never via
rank-dependent addressing.  Output: per-core partial sum of squared error
over its channel slice; the host sums the 8 partials and divides (unshard).
"""

import os
import sys
import types

import numpy as np

B, N, TC, TF, HG, L = 8, 2048, 96, 64, 64, 4
STEPS = 100
R = 8                 # cores
S = N // R            # 256 channels per core
NCH = N // 128        # 16 chunks of 128 channels
FBT = B * TF          # 512 = (b, t) free layout
PAD = 16              # left zero-pad per batch block (= (K-1)*max_dilation)
TPD = TF + PAD        # 80


def _alphas_bar(T=STEPS, s=0.008):
    t = np.linspace(0.0, T, T + 1)
    f = np.cos((t / T + s) / (1 + s) * np.pi / 2) ** 2
    ab = f / f[0]
    betas = np.clip(1.0 - ab[1:] / ab[:-1], 1e-6, 0.999)
    return np.cumprod(1.0 - betas).astype(np.float32)


_ALPHAS_BAR = _alphas_bar()

# ---------------------------------------------------------------------------
# runtime shims: NTFF profile hook glue + Tile fixes for the neuronxcc CoreV3
# codegen (one semaphore wait per instruction)
# ---------------------------------------------------------------------------

_ENV_READY = False


def _setup_env():
    global _ENV_READY
    if _ENV_READY:
        return
    import antenv

    if "antenv.axon_hooks" not in sys.modules:
        hooks_mod = types.ModuleType("antenv.axon_hooks")
        _hook = [None]
        hooks_mod.set_axon_ntff_profile_hook = lambda h: _hook.__setitem__(0, h)
        hooks_mod.get_axon_ntff_profile_hook = lambda: _hook[0]
        sys.modules["antenv.axon_hooks"] = hooks_mod
        antenv.axon_hooks = hooks_mod
        try:
            from trn_agent_boot.trn_boot import _ntff_profile_via_ctypes

            hooks_mod.set_axon_ntff_profile_hook(
                _ntff_profile_via_ctypes("/opt/axon/libaxon_pjrt.so")
            )
        except Exception:
            pass

    import concourse.bass_utils as bass_utils

    bass_utils.upload_artifacts = lambda tmpdir: f"file://{tmpdir}"

    import concourse.mybir as mybir
    from concourse import tile
    from bass_rust import ScopedClock

    def _drain_and_barrier(self, tick_clock, wait_clock):
        drain_inst = self.nc.sync.drain()
        wait_clock.add_sem_waits(
            drain_inst.ins, ScopedClock({None: tick_clock.global_clock})
        )
        si = drain_inst.ins.sync_info
        if si is not None and len(si.on_wait) > 1:
            waits = list(si.on_wait)
            upd = list(si.on_update)
            drain_inst.ins.sync_info = mybir.SyncInfo(
                on_wait=[waits[0]], on_update=upd
            )
            for w in waits[1:]:
                nop = self.nc.sync.nop(nofuse=True, hint="drain_split")
                nop.ins.sync_info = mybir.SyncInfo(on_wait=[w], on_update=[])
        self.nc.all_engine_barrier()
        assert self.sems is not None
        popped = self.nc._tile_sem_poison_stack.pop()
        assert popped is self._sem_poison
        self.nc.clear_and_free_semaphores(list(self.sems.allocated().values()))
        self.nc.all_engine_barrier()

    tile.TileContext._drain_and_barrier = _drain_and_barrier
    _ENV_READY = True


def _split_waits(nc, maxw=1):
    import concourse.mybir as mybir

    cnt = 0
    for fn in nc.m.functions:
        for bb in fn.blocks:
            insts = bb.instructions
            i = 0
            while i < len(insts):
                inst = insts[i]
                si = inst.sync_info
                if si is not None and len(si.on_wait) > maxw:
                    waits = list(si.on_wait)
                    inst.sync_info = mybir.SyncInfo(
                        on_wait=waits[:maxw], on_update=list(si.on_update)
                    )
                    for w in waits[maxw:]:
                        cnt += 1
                        nop = mybir.InstNoOp(
                            name=f"waitsplit_{cnt}",
                            engine=inst.engine,
                            sync_info=mybir.SyncInfo(on_wait=[w], on_update=[]),
                        )
                        insts.insert(i, nop)
                        i += 1
                i += 1
    return cnt


# ---------------------------------------------------------------------------
# the Bass program (identical on every core)
# ---------------------------------------------------------------------------

_CACHE = {}


def _build_program():
    import concourse.bass as bass
    import concourse.mybir as mybir
    from concourse import tile

    f32 = mybir.dt.float32
    f32r = mybir.dt.float32r
    bf16 = mybir.dt.bfloat16
    AF = mybir.ActivationFunctionType
    ALU = mybir.AluOpType

    nc = bass.Bass(num_devices=R)

    def din(name, shape):
        return nc.dram_tensor(name, list(shape), f32, kind="ExternalInput")

    ctx_t = din("ctx_t", (N, TC * B))
    fut_t = din("fut_t", (N, FBT))
    noise_t = din("noise_t", (N, FBT))
    s0m = din("s0m", (128, FBT))
    s1m = din("s1m", (128, FBT))
    futs = din("futs", (S, FBT))
    noises = din("noises", (S, FBT))
    convw_t = din("convw_t", (L, 2, 128, NCH * 3 * 128))
    convb_t = din("convb_t", (128, L * 2))
    projw_t = din("projw_t", (L, 128, NCH * 2 * 128))
    projb_t = din("projb_t", (128, L * 2))
    outw_t = din("outw_t", (N, S))
    outb_t = din("outb_t", (128, 2))
    gatw_o = din("gatw_o", (TF, TF))
    gatw_tr = din("gatw_tr", (TF, TF))
    gata_t = din("gata_t", (TF, 2))
    htpw_t = din("htpw_t", (HG, S))
    htpb_t = din("htpb_t", (128, 2))
    gruw_t = din("gruw_t", (N, 3 * HG))
    gruu_t = din("gruu_t", (HG, 3 * HG))
    grub_t = din("grub_t", (HG, 6))
    ident = din("ident", (128, 128))
    onesk = din("onesk", (1, S))
    zpad = din("zpad", (128, NCH * B * PAD))

    h_in = [
        [nc.dram_tensor(f"h_in{l}_{m}", [128, FBT], bf16) for m in range(2)]
        for l in range(L)
    ]
    h_out = [
        [
            nc.dram_tensor(
                f"h_out{l}_{m}", [128 * R, FBT], bf16, addr_space="Shared"
            )
            for m in range(2)
        ]
        for l in range(L)
    ]
    blk_in = [
        [nc.dram_tensor(f"blk_in{l}_{m}", [128, FBT], bf16) for m in range(2)]
        for l in range(L)
    ]
    blk_out = [
        [
            nc.dram_tensor(
                f"blk_out{l}_{m}", [128 * R, FBT], bf16, addr_space="Shared"
            )
            for m in range(2)
        ]
        for l in range(L)
    ]
    warm_in = nc.dram_tensor("warm_in", [128, 2], f32)
    warm_out = nc.dram_tensor("warm_out", [128 * R, 2], f32, addr_space="Shared")
    y_in = nc.dram_tensor("y_in", [S, FBT], bf16)
    y_out = nc.dram_tensor("y_out", [N, FBT], bf16, addr_space="Shared")
    mse_part = nc.dram_tensor("mse_part", [1, 1], f32, kind="ExternalOutput")

    RG = [list(range(R))]

    with tile.TileContext(nc) as tc, \
         tc.tile_pool(name="consts", bufs=1) as cpool, \
         tc.tile_pool(name="big", bufs=1) as big, \
         tc.tile_pool(name="wts", bufs=2) as wpool, \
         tc.tile_pool(name="pwp", bufs=1) as pwpool, \
         tc.tile_pool(name="stream", bufs=3) as spool, \
         tc.tile_pool(name="psA", bufs=4, space="PSUM") as psA, \
         tc.tile_pool(name="psS", bufs=2, space="PSUM") as psS, \
         tc.tile_pool(name="psRZ", bufs=1, space="PSUM") as psRZ, \
         tc.tile_pool(name="psN", bufs=1, space="PSUM") as psN:

        # ------------------------ constants ------------------------
        ident_sb = cpool.tile([128, 128], f32r)
        nc.sync.dma_start(out=ident_sb[:], in_=ident[:].bitcast(f32r))
        onesk_sb = cpool.tile([1, S], f32r)
        nc.sync.dma_start(out=onesk_sb[:], in_=onesk[:].bitcast(f32r))
        s0_sb = cpool.tile([128, FBT], f32)
        nc.sync.dma_start(out=s0_sb[:], in_=s0m[:])
        s1_sb = cpool.tile([128, FBT], f32)
        nc.sync.dma_start(out=s1_sb[:], in_=s1m[:])
        convb_sb = cpool.tile([128, L * 2], f32)
        nc.sync.dma_start(out=convb_sb[:], in_=convb_t[:])
        projb_sb = cpool.tile([128, L * 2], f32)
        nc.sync.dma_start(out=projb_sb[:], in_=projb_t[:])
        outb_sb = cpool.tile([128, 2], f32)
        nc.sync.dma_start(out=outb_sb[:], in_=outb_t[:])
        gatw_o_sb = cpool.tile([TF, TF], f32r)
        nc.sync.dma_start(out=gatw_o_sb[:], in_=gatw_o[:].bitcast(f32r))
        gatw_tr_sb = cpool.tile([TF, TF], f32r)
        nc.sync.dma_start(out=gatw_tr_sb[:], in_=gatw_tr[:].bitcast(f32r))
        gata_sb = cpool.tile([TF, 2], f32r)
        nc.sync.dma_start(out=gata_sb[:], in_=gata_t[:].bitcast(f32r))
        htpw_sb = cpool.tile([HG, S], f32r)
        nc.sync.dma_start(out=htpw_sb[:], in_=htpw_t[:].bitcast(f32r))
        htpb_sb = cpool.tile([128, 2], f32)
        nc.sync.dma_start(out=htpb_sb[:], in_=htpb_t[:])
        gruu_sb = cpool.tile([HG, 3 * HG], f32r)
        nc.sync.dma_start(out=gruu_sb[:], in_=gruu_t[:].bitcast(f32r))
        grub_sb = cpool.tile([HG, 6], f32)
        nc.sync.dma_start(out=grub_sb[:], in_=grub_t[:])
        zero_h = cpool.tile([HG, B], f32)
        nc.vector.memset(zero_h[:], 0.0)
        zcol = cpool.tile([128, NCH], f32)
        nc.vector.memset(zcol[:], 0.0)

        # collective warmup: tiny AG so ncfw init overlaps the early compute
        wtile = cpool.tile([128, 2], f32)
        nc.vector.memset(wtile[:], 0.0)
        nc.sync.dma_start(out=warm_in[:], in_=wtile[:])
        nc.gpsimd.collective_compute(
            "AllGather", ALU.bypass, ins=[warm_in[:]], outs=[warm_out[:]],
            replica_groups=RG,
        )

        # state tiles
        hT = cpool.tile([HG, B], f32r)          # GRU hidden, transposed [h, b]
        gi_sb = big.tile([HG, TC * 3 * B], f32r)  # [h, (s, g, b)]
        gi_v = gi_sb[:].rearrange("p (s g b) -> p s g b", g=3, b=B)
        condT = cpool.tile([128, 2, B], f32)
        ypad_full = big.tile([128, NCH * B * TPD + 2], f32r)
        ypad = ypad_full[:, 0:NCH * B * TPD].rearrange(
            "p (c b t) -> p c b t", c=NCH, b=B
        )
        y_slice = big.tile([128, 2, FBT], f32r)
        noises_sb = big.tile([128, 2, FBT], f32)
        nc.sync.dma_start(
            out=noises_sb[:], in_=noises[:].rearrange("(m p) f -> p m f", p=128)
        )
        Ysl = big.tile([128, 2, FBT], bf16)

        # ==========================================================
        # Phase 0: GRU input projection + recurrence (high priority,
        # overlaps everything)
        # ==========================================================
        if True:
            nc.vector.tensor_copy(hT[:], zero_h[:])

            with tc.tile_pool(name="ctxp", bufs=1) as cxp:
                gruw_sb = cxp.tile([128, NCH, 3 * HG], f32r)
                nc.sync.dma_start(
                    out=gruw_sb[:],
                    in_=gruw_t[:].bitcast(f32r).rearrange("(c p) f -> p c f", p=128),
                )
                gi_ps = []
                for g in range(3):
                    for half in range(2):
                        tag = "mm" if len(gi_ps) < 4 else "sm"
                        pool_ = psA if tag == "mm" else psS
                        gi_ps.append(
                            pool_.tile(
                                [HG, 48, B], f32, tag=tag,
                                name=f"gi_ps{g}_{half}",
                            )
                        )
                for ch in range(2):
                    ctxh = cxp.tile(
                        [128, 8, TC * B], f32r, tag="ctxh", name=f"ctxh{ch}"
                    )
                    nc.sync.dma_start(
                        out=ctxh[:],
                        in_=ctx_t[ch * 1024:(ch + 1) * 1024, :]
                        .bitcast(f32r)
                        .rearrange("(c p) f -> p c f", p=128),
                    )
                    for g in range(3):
                        for half in range(2):
                            ps_gi = gi_ps[g * 2 + half]
                            for cc in range(8):
                                nc.tensor.matmul(
                                    ps_gi[:],
                                    gruw_sb[:, ch * 8 + cc, g * HG:(g + 1) * HG],
                                    ctxh[:, cc, half * 384:(half + 1) * 384],
                                    start=(ch == 0 and cc == 0),
                                    stop=(ch == 1 and cc == 7),
                                )
                for g in range(3):
                    for half in range(2):
                        nc.vector.tensor_copy(
                            gi_v[:, half * 48:(half + 1) * 48, g, :],
                            gi_ps[g * 2 + half][:],
                        )
            # fold biases: r,z get bih+bhh; n gets bih only (bhh_n per step)
            for g in range(3):
                if g < 2:
                    nc.vector.tensor_scalar(
                        out=gi_v[:, :, g, :],
                        in0=gi_v[:, :, g, :],
                        scalar1=grub_sb[:, g:g + 1],
                        scalar2=grub_sb[:, 3 + g:4 + g],
                        op0=ALU.add,
                        op1=ALU.add,
                    )
                else:
                    nc.vector.tensor_scalar(
                        out=gi_v[:, :, g, :],
                        in0=gi_v[:, :, g, :],
                        scalar1=grub_sb[:, g:g + 1],
                        scalar2=None,
                        op0=ALU.add,
                    )

        gru_state = {"s": 0}

        def emit_gru_steps(n):
            for _ in range(n):
                s = gru_state["s"]
                if s >= TC:
                    return
                gru_state["s"] += 1
                ps_rz = psRZ.tile([HG, 2, B], f32, tag="rz", name=f"ps_rz{s}")
                # off-chain: preload gi (with biases) via PE copy-matmul so the
                # rz accumulation lands gh+gi directly in PSUM
                nc.tensor.matmul(
                    ps_rz[:],
                    ident_sb[0:HG, 0:HG],
                    gi_v[:, s, 0:2, :],
                    start=True,
                    stop=False,
                )
                nc.tensor.matmul(
                    ps_rz[:, 0, :], gruu_sb[:, 0:HG], hT[:],
                    start=False, stop=False,
                )
                nc.tensor.matmul(
                    ps_rz[:, 1, :], gruu_sb[:, HG:2 * HG], hT[:],
                    start=False, stop=True,
                )
                ps_n = psN.tile([HG, B], f32, tag="n", name=f"ps_n{s}")
                nc.tensor.matmul(
                    ps_n[:], gruu_sb[:, 2 * HG:3 * HG], hT[:],
                    start=True, stop=True,
                )
                rz = spool.tile([HG, 2, B], f32, tag="gr_rz")
                nc.scalar.activation(rz[:], ps_rz[:], AF.Sigmoid)
                t3 = spool.tile([HG, B], f32, tag="gr_t3")
                nc.vector.scalar_tensor_tensor(
                    out=t3[:], in0=ps_n[:], scalar=grub_sb[:, 5:6],
                    in1=rz[:, 0, :], op0=ALU.add, op1=ALU.mult,
                )
                t4 = spool.tile([HG, B], f32, tag="gr_t4")
                nc.vector.tensor_tensor(t4[:], t3[:], gi_v[:, s, 2, :], ALU.add)
                nn_ = spool.tile([HG, B], f32, tag="gr_n")
                nc.scalar.activation(nn_[:], t4[:], AF.Tanh)
                d_ = spool.tile([HG, B], f32, tag="gr_d")
                nc.vector.tensor_tensor(d_[:], hT[:], nn_[:], ALU.subtract)
                e_ = spool.tile([HG, B], f32, tag="gr_e")
                nc.vector.tensor_tensor(e_[:], d_[:], rz[:, 1, :], ALU.mult)
                nc.vector.tensor_tensor(hT[:], nn_[:], e_[:], ALU.add)

        # ==========================================================
        # Phase 1: xk = sqrt(ab)*fut + sqrt(1-ab)*noise  ->  ypad, y_slice
        # ==========================================================
        nc.sync.dma_start(
            out=ypad[:, :, :, 0:PAD],
            in_=zpad[:].bitcast(f32r).rearrange(
                "p (c b t) -> p c b t", c=NCH, b=B
            ),
        )
        with tc.tile_pool(name="xkp", bufs=2) as xkp:
            for q in range(8):
                fr = xkp.tile([128, 2, FBT], f32, tag="fr")
                nc.sync.dma_start(
                    out=fr[:],
                    in_=fut_t[q * 256:(q + 1) * 256, :].rearrange(
                        "(c p) f -> p c f", p=128
                    ),
                )
                nr = xkp.tile([128, 2, FBT], f32, tag="nr")
                nc.sync.dma_start(
                    out=nr[:],
                    in_=noise_t[q * 256:(q + 1) * 256, :].rearrange(
                        "(c p) f -> p c f", p=128
                    ),
                )
                for cc in range(2):
                    c = q * 2 + cc
                    t0 = xkp.tile([128, FBT], f32, tag="t0")
                    nc.vector.tensor_tensor(t0[:], fr[:, cc, :], s0_sb[:], ALU.mult)
                    t1x = xkp.tile([128, FBT], f32, tag="t1x")
                    nc.vector.tensor_tensor(
                        t1x[:], nr[:, cc, :], s1_sb[:], ALU.mult
                    )
                    nc.vector.tensor_tensor(
                        ypad[:, c, :, PAD:],
                        t0[:].rearrange("p (b t) -> p b t", b=B),
                        t1x[:].rearrange("p (b t) -> p b t", b=B),
                        ALU.add,
                    )
                emit_gru_steps(1)
            fs = xkp.tile([128, 2, FBT], f32, tag="fs")
            nc.sync.dma_start(
                out=fs[:], in_=futs[:].rearrange("(m p) f -> p m f", p=128)
            )
            for m in range(2):
                t0 = xkp.tile([128, FBT], f32, tag="t0")
                nc.vector.tensor_tensor(t0[:], fs[:, m, :], s0_sb[:], ALU.mult)
                t1x = xkp.tile([128, FBT], f32, tag="t1x")
                nc.vector.tensor_tensor(
                    t1x[:], noises_sb[:, m, :], s1_sb[:], ALU.mult
                )
                nc.vector.tensor_tensor(y_slice[:, m, :], t0[:], t1x[:], ALU.add)

        # ==========================================================
        # Phase 2: temporal layers, m-half pipelined with bf16 AllGathers
        # ==========================================================
        EVENS = list(range(0, NCH, 2))
        ODDS = list(range(1, NCH, 2))
        with tc.tile_pool(name="hfp", bufs=1) as hfp:
            hfull = hfp.tile([128, NCH, FBT], bf16)
            for l in range(L):
                dil = 2 ** l
                # --- conv, one m output-half at a time (staggers h AGs) ---
                for m in range(2):
                    cwh = [None, None]
                    for hh in range(2):
                        cwh[hh] = wpool.tile(
                            [128, 8, 3, 128], f32r, tag="convw",
                            name=f"cw{l}_{m}_{hh}",
                        )
                        nc.scalar.dma_start(
                            out=cwh[hh][:],
                            in_=convw_t[l, m][
                                :, hh * 8 * 384:(hh + 1) * 8 * 384
                            ]
                            .bitcast(f32r)
                            .rearrange("p (c k o) -> p c k o", c=8, k=3),
                        )
                    ps_h = psA.tile(
                        [128, B, TF], f32, tag="mm", name=f"ps_h{l}_{m}"
                    )
                    for ci, c in enumerate(EVENS + ODDS):
                        for k in range(3):
                            off = PAD - (2 - k) * dil
                            nc.tensor.matmul(
                                ps_h[:],
                                cwh[c // 8][:, c % 8, k, :],
                                ypad[:, c, :, off:off + TF],
                                start=(ci == 0 and k == 0),
                                stop=(ci == NCH - 1 and k == 2),
                            )
                    hmy = spool.tile([128, B, TF], bf16, tag="hmy")
                    nc.vector.tensor_scalar(
                        out=hmy[:],
                        in0=ps_h[:],
                        scalar1=convb_sb[:, l * 2 + m:l * 2 + m + 1],
                        scalar2=0.0,
                        op0=ALU.add,
                        op1=ALU.max,
                    )
                    nc.sync.dma_start(
                        out=h_in[l][m][:],
                        in_=hmy[:].rearrange("p b t -> p (b t)"),
                    )
                    nc.gpsimd.collective_compute(
                        "AllGather",
                        ALU.bypass,
                        ins=[h_in[l][m][:]],
                        outs=[h_out[l][m][:]],
                        replica_groups=RG,
                    )
                    emit_gru_steps(5)
                    nc.sync.dma_start(
                        out=hfull[:, m::2, :],
                        in_=h_out[l][m][:].rearrange(
                            "(rr p) f -> p rr f", p=128
                        ),
                    )
                # --- proj, chunk-parity ordered (even chunks arrive first) ---
                ps_b = [
                    psA.tile([128, FBT], f32, tag="mm", name=f"ps_b{l}_{i}")
                    for i in range(2)
                ]
                pw = pwpool.tile([128, NCH, 2, 128], bf16, tag="projw")
                nc.gpsimd.dma_start(
                    out=pw[:],
                    in_=projw_t[l].rearrange(
                        "p (c md o) -> p c md o", c=NCH, md=2
                    ),
                )
                first = [True, True]
                for ci, c in enumerate(EVENS + ODDS):
                    for md in range(2):
                        nc.tensor.matmul(
                            ps_b[md][:],
                            pw[:, c, md, :],
                            hfull[:, c, :],
                            start=first[md],
                            stop=(ci == NCH - 1),
                        )
                        first[md] = False
                emit_gru_steps(4)
                for md in range(2):
                    blk = spool.tile([128, FBT], bf16, tag="blk")
                    nc.vector.tensor_scalar(
                        out=blk[:],
                        in0=ps_b[md][:],
                        scalar1=projb_sb[:, l * 2 + md:l * 2 + md + 1],
                        scalar2=None,
                        op0=ALU.add,
                    )
                    nc.sync.dma_start(out=blk_in[l][md][:], in_=blk[:])
                    nc.vector.tensor_tensor(
                        y_slice[:, md, :], y_slice[:, md, :], blk[:], ALU.add
                    )
                    nc.gpsimd.collective_compute(
                        "AllGather",
                        ALU.bypass,
                        ins=[blk_in[l][md][:]],
                        outs=[blk_out[l][md][:]],
                        replica_groups=RG,
                    )
                    emit_gru_steps(2)
                for m in range(2):
                    bfm = spool.tile([128, R, FBT], bf16, tag="bf")
                    nc.sync.dma_start(
                        out=bfm[:],
                        in_=blk_out[l][m][:].rearrange("(rr p) f -> p rr f", p=128),
                    )
                    nc.vector.tensor_tensor(
                        ypad[:, m::2, :, PAD:],
                        ypad[:, m::2, :, PAD:],
                        bfm[:].rearrange("p rr (b t) -> p rr b t", b=B),
                        ALU.add,
                    )

        # q0/q1 = gat_w.T @ gat_a halves
        ps_q = psS.tile([TF, 2], f32, tag="sm")
        nc.tensor.matmul(ps_q[:], gatw_o_sb[:], gata_sb[:], start=True, stop=True)
        q01_sb = cpool.tile([TF, 2], f32r)
        nc.vector.tensor_copy(q01_sb[:], ps_q[:])
        # q1 as a [1, 64] row then broadcast to q1_mat [128, 64]
        ps_q1r = psS.tile([1, TF], f32r, tag="sm")
        nc.tensor.transpose(ps_q1r[:], q01_sb[:, 1:2], ident_sb[0:TF, 0:TF])
        q1row = cpool.tile([1, TF], f32r)
        nc.vector.tensor_copy(q1row[:], ps_q1r[:])
        ps_q1m = psS.tile([128, TF], f32, tag="sm")
        nc.tensor.matmul(
            ps_q1m[:], onesk_sb[:, 0:128], q1row[:], start=True, stop=True
        )
        q1_mat = cpool.tile([128, TF], f32r)
        nc.vector.tensor_copy(q1_mat[:], ps_q1m[:])
        # ones marker at each batch-block's first pad column (read as the
        # 65th lhsT column of the previous block's V matmul) + tail cells
        nc.vector.tensor_scalar(
            out=ypad[:, :, :, 0:1].rearrange("p c b o -> p (c b o)"),
            in0=ident_sb[:],
            scalar1=0.0,
            scalar2=1.0,
            op0=ALU.mult,
            op1=ALU.add,
        )
        nc.vector.tensor_scalar(
            out=ypad_full[:, NCH * B * TPD:NCH * B * TPD + 2],
            in0=ident_sb[:, 0:2],
            scalar1=0.0,
            scalar2=1.0,
            op0=ALU.mult,
            op1=ALU.add,
        )

        # ==========================================================
        # Phase 4: GAT  (V = softmax-numerator @ y, then @ gat_w.T)
        # ==========================================================
        with tc.tile_pool(name="gat", bufs=1) as gpool:
            for b in range(B):
                yTs = gpool.tile([TF, S], f32r, tag="yTs")
                for m in range(2):
                    ps_t = psS.tile([TF, 128], f32r, tag="sm")
                    nc.tensor.transpose(
                        ps_t[:], y_slice[:, m, b * TF:(b + 1) * TF], ident_sb[:]
                    )
                    nc.vector.tensor_copy(yTs[:, m * 128:(m + 1) * 128], ps_t[:])

                ps_ei = psS.tile([1, S], f32, tag="sm")
                nc.tensor.matmul(
                    ps_ei[:], q01_sb[:, 0:1], yTs[:], start=True, stop=True
                )
                ei_row = gpool.tile([1, S], f32r, tag="eirow")
                nc.vector.tensor_copy(ei_row[:], ps_ei[:])
                ps_EI = psS.tile([128, S], f32, tag="sm")
                nc.tensor.matmul(
                    ps_EI[:], onesk_sb[:, 0:128], ei_row[:], start=True, stop=True
                )
                EI_sb = gpool.tile([128, S], f32, tag="EI")
                nc.scalar.activation(EI_sb[:], ps_EI[:], AF.Copy)

                ejT = gpool.tile([128, NCH], f32, tag="ejT")
                for c in range(NCH):
                    ttr_scr = spool.tile([128, TF], f32, tag="ttr")
                    nc.vector.scalar_tensor_tensor(
                        out=ttr_scr[:],
                        in0=ypad[:, c, b, PAD:],
                        scalar=1.0,
                        in1=q1_mat[:],
                        op0=ALU.mult,
                        op1=ALU.mult,
                        accum_out=ejT[:, c:c + 1],
                    )

                expe = gpool.tile([128, NCH, S], f32r, tag="expe")
                for c in range(NCH):
                    lr = spool.tile([128, S], f32, tag="lr")
                    nc.scalar.activation(
                        lr[:], EI_sb[:], AF.Prelu,
                        bias=ejT[:, c:c + 1], alpha=0.2,
                    )
                    nc.scalar.activation(expe[:, c, :], lr[:], AF.Exp)

                emit_gru_steps(2)
                ps_v = psA.tile([TF + 1, S], f32, tag="mm")
                for c in range(NCH):
                    off = (c * B + b) * TPD + PAD
                    nc.tensor.matmul(
                        ps_v[:],
                        ypad_full[:, off:off + TF + 1],
                        expe[:, c, :],
                        start=(c == 0),
                        stop=(c == NCH - 1),
                    )
                v_sb = gpool.tile([TF + 1, S], f32r, tag="vsb")
                nc.vector.tensor_copy(v_sb[:], ps_v[:])
                ps_u2 = psS.tile([TF, S], f32, tag="sm")
                nc.tensor.matmul(
                    ps_u2[:], gatw_tr_sb[:], v_sb[0:TF, :],
                    start=True, stop=True,
                )
                u_sb = gpool.tile([TF, S], f32r, tag="usb")
                nc.vector.tensor_copy(u_sb[:], ps_u2[:])
                for m in range(2):
                    ps_st = psS.tile([128, 2], f32r, tag="sm")
                    nc.tensor.transpose(
                        ps_st[:], v_sb[TF:TF + 1, m * 128:(m + 1) * 128],
                        ident_sb[TF:TF + 1, TF:TF + 2],
                    )
                    invS = spool.tile([128, 1], f32, tag="invs")
                    nc.vector.reciprocal(invS[:], ps_st[:, 0:1])
                    ps_y = psS.tile([128, TF], f32r, tag="sm")
                    nc.tensor.transpose(
                        ps_y[:], u_sb[:, m * 128:(m + 1) * 128],
                        ident_sb[0:TF, 0:TF],
                    )
                    nc.vector.tensor_scalar(
                        out=Ysl[:, m, b * TF:(b + 1) * TF],
                        in0=ps_y[:],
                        scalar1=invS[:],
                        scalar2=None,
                        op0=ALU.mult,
                    )

        emit_gru_steps(TC)
        for m in range(2):
            ps_c = psS.tile([128, B], f32, tag="sm")
            nc.tensor.matmul(
                ps_c[:], htpw_sb[:, m * 128:(m + 1) * 128], hT[:],
                start=True, stop=True,
            )
            nc.vector.tensor_scalar(
                out=condT[:, m, :], in0=ps_c[:],
                scalar1=htpb_sb[:, m:m + 1], scalar2=None, op0=ALU.add,
            )

        for m in range(2):
            for b in range(B):
                nc.vector.tensor_scalar(
                    out=Ysl[:, m, b * TF:(b + 1) * TF],
                    in0=Ysl[:, m, b * TF:(b + 1) * TF],
                    scalar1=condT[:, m, b:b + 1],
                    scalar2=None,
                    op0=ALU.add,
                )
        nc.sync.dma_start(
            out=y_in[:].rearrange("(m p) f -> p m f", p=128), in_=Ysl[:]
        )
        nc.gpsimd.collective_compute(
            "AllGather", ALU.bypass, ins=[y_in[:]], outs=[y_out[:]],
            replica_groups=RG,
        )

        # ==========================================================
        # Phase 5: eps = out_w @ Y + out_b ; partial MSE
        # ==========================================================
        ps_eps = [
            psA.tile([128, FBT], f32, tag="mm", name=f"ps_eps{i}") for i in range(2)
        ]
        oww = big.tile([128, NCH, S], bf16, tag="outw")
        nc.gpsimd.dma_start(
            out=oww[:],
            in_=outw_t[:].rearrange("(c p) s -> p c s", p=128),
        )
        for q in range(4):
            yf = spool.tile([128, 4, FBT], bf16, tag="yf")
            nc.sync.dma_start(
                out=yf[:],
                in_=y_out[q * 512:(q + 1) * 512, :].rearrange(
                    "(c p) f -> p c f", p=128
                ),
            )
            for cc in range(4):
                c = q * 4 + cc
                for m in range(2):
                    nc.tensor.matmul(
                        ps_eps[m][:],
                        oww[:, c, m * 128:(m + 1) * 128],
                        yf[:, cc, :],
                        start=(c == 0),
                        stop=(c == NCH - 1),
                    )
        macc = cpool.tile([128, 2], f32)
        for m in range(2):
            dd = spool.tile([128, FBT], f32, tag="dd")
            nc.vector.tensor_tensor(
                dd[:], ps_eps[m][:], noises_sb[:, m, :], ALU.subtract
            )
            scrap = spool.tile([128, FBT], f32, tag="scrap")
            nc.scalar.activation(
                scrap[:], dd[:], AF.Square,
                bias=outb_sb[:, m:m + 1], accum_out=macc[:, m:m + 1],
            )
        msum = cpool.tile([128, 1], f32r)
        nc.vector.tensor_tensor(
            msum[:], macc[:, 0:1], macc[:, 1:2], ALU.add
        )
        ps_mt = psS.tile([1, 128], f32r, tag="sm")
        nc.tensor.transpose(ps_mt[:], msum[:], ident_sb[:])
        mred = cpool.tile([1, 1], f32)
        nc.vector.tensor_reduce(
            out=mred[:], in_=ps_mt[:], axis=mybir.AxisListType.X, op=ALU.add
        )
        nc.sync.dma_start(out=mse_part[:], in_=mred[:])

    _split_waits(nc)
    return nc


# ---------------------------------------------------------------------------
# host side: shard/layout inputs, run, unshard
# ---------------------------------------------------------------------------


def _prep_inputs(inputs):
    f = np.float32
    ctx = np.asarray(inputs["ctx"], f)
    fut = np.asarray(inputs["fut"], f)
    noise = np.asarray(inputs["noise"], f)
    conv_w = np.asarray(inputs["conv_w"], f)
    conv_b = np.asarray(inputs["conv_b"], f)
    proj_w = np.asarray(inputs["proj_w"], f)
    proj_b = np.asarray(inputs["proj_b"], f)
    gat_w = np.asarray(inputs["gat_w"], f)
    gat_a = np.asarray(inputs["gat_a"], f)
    out_w = np.asarray(inputs["out_w"], f)
    out_b = np.asarray(inputs["out_b"], f)
    htp_w = np.asarray(inputs["htp_w"], f)
    htp_b = np.asarray(inputs["htp_b"], f)
    wih = np.asarray(inputs["gru_wih"], f)
    whh = np.asarray(inputs["gru_whh"], f)
    bih = np.asarray(inputs["gru_bih"], f)
    bhh = np.asarray(inputs["gru_bhh"], f)
    k = np.asarray(inputs["k"])  # int32, consumed host-side (table lookup)

    ab = _ALPHAS_BAR[k]
    s0 = np.sqrt(ab).astype(f)
    s1 = np.sqrt(1.0 - ab).astype(f)
    s0v = np.repeat(s0, TF)[None, :]
    s1v = np.repeat(s1, TF)[None, :]
    s0m = np.ascontiguousarray(np.broadcast_to(s0v, (128, FBT)))
    s1m = np.ascontiguousarray(np.broadcast_to(s1v, (128, FBT)))

    ctx_t = np.ascontiguousarray(ctx.transpose(1, 2, 0).reshape(N, TC * B))
    fut_t = np.ascontiguousarray(fut.transpose(1, 0, 2).reshape(N, FBT))
    noise_t = np.ascontiguousarray(noise.transpose(1, 0, 2).reshape(N, FBT))
    gata_t = np.ascontiguousarray(np.stack([gat_a[:TF], gat_a[TF:]], 1))
    gruw_t = np.ascontiguousarray(wih.T)
    gruu_t = np.ascontiguousarray(whh.T)
    grub_t = np.ascontiguousarray(
        np.concatenate([bih.reshape(3, HG).T, bhh.reshape(3, HG).T], 1)
    )
    ident = np.eye(128, dtype=f)
    onesk = np.ones((1, S), f)
    zpad = np.zeros((128, NCH * B * PAD), f)

    shared = dict(
        ctx_t=ctx_t, fut_t=fut_t, noise_t=noise_t, s0m=s0m, s1m=s1m,
        gatw_o=np.ascontiguousarray(gat_w),
        gatw_tr=np.ascontiguousarray(gat_w.T),
        gata_t=gata_t, gruw_t=gruw_t, gruu_t=gruu_t, grub_t=grub_t,
        ident=ident, onesk=onesk, zpad=zpad,
    )

    in_maps = []
    for r in range(R):
        rs, re = r * S, (r + 1) * S
        m = dict(shared)
        m["futs"] = np.ascontiguousarray(fut_t[rs:re, :])
        m["noises"] = np.ascontiguousarray(noise_t[rs:re, :])
        m["convw_t"] = np.ascontiguousarray(
            conv_w[:, rs:re]
            .reshape(L, 2, 128, NCH, 128, 3)
            .transpose(0, 1, 4, 3, 5, 2)
            .reshape(L, 2, 128, NCH * 3 * 128)
        )
        m["convb_t"] = np.ascontiguousarray(
            conv_b[:, rs:re].reshape(L, 2, 128).transpose(2, 0, 1).reshape(128, L * 2)
        )
        m["projw_t"] = np.ascontiguousarray(
            proj_w[:, rs:re]
            .reshape(L, 2, 128, NCH, 128)
            .transpose(0, 4, 3, 1, 2)
            .reshape(L, 128, NCH * 2 * 128)
        )
        m["projb_t"] = np.ascontiguousarray(
            proj_b[:, rs:re].reshape(L, 2, 128).transpose(2, 0, 1).reshape(128, L * 2)
        )
        m["outw_t"] = np.ascontiguousarray(out_w[rs:re, :].T)
        m["outb_t"] = np.ascontiguousarray(out_b[rs:re].reshape(2, 128).T)
        m["htpw_t"] = np.ascontiguousarray(htp_w[rs:re, :].T)
        m["htpb_t"] = np.ascontiguousarray(htp_b[rs:re].reshape(2, 128).T)
        in_maps.append(m)
    return in_maps


def kernel(**inputs):
    _setup_env()
    from concourse.bass_utils import run_bass_kernel_spmd

    if "nc" not in _CACHE:
        _CACHE["nc"] = _build_program()
    nc = _CACHE["nc"]

    in_maps = _prep_inputs(inputs)
    trace = os.environ.get("BASS_KERNEL_TRACE", "0") == "1"
    res = run_bass_kernel_spmd(nc, in_maps, list(range(R)), trace=trace)
    if trace and res.exec_time_ns is not None:
        print(f"HW exec time: {res.exec_time_ns} ns")
        _CACHE["exec_time_ns"] = res.exec_time_ns
        _CACHE["profile_json"] = res.profile_json

    total = 0.0
    for r in range(R):
        total += float(res.results[r]["mse_part"][0, 0])
    return np.asarray(total / (B * N * TF), dtype=np.float32)

